# revision 34
# baseline (speedup 1.0000x reference)
"""Multi-head attention (B=2, S=2048, D=1024, H=16) on 8 TRN2 NeuronCores.

Sharding: core c handles batch b = c // 4 and heads 4*(c%4) .. 4*(c%4)+3.
Each core computes its 4 heads' Q/K/V projections, head-local attention,
and a partial output projection (row slice of Wo). Host sums the partials.

Key points vs the fp16 baseline:
- Everything bf16. Scores are computed pre-scaled by 128/ln2 (folded into
  Wq host-side) so both exp engines work in "bf16 index space".
- Softmax exp runs on TWO engines: ScalarE does true exp (free affine
  bias/scale) while the DVE runs a custom 8-slice op that emits the
  bf16 BIT PATTERN of exp(x)*2^-1/2 in the low half of an fp32 write
  (floor-split at the 128-index grain + quadratic mantissa poly). The
  global 2^-1/2 factor is softmax-invariant. The PV matmul reads DVE
  tiles through a stride-2 bf16 view.
- Score matmuls (contraction=64) are issued as back-to-back even/odd-head
  pairs on PE row tiles (0,0)/(64,0) writing different PSUM banks, so the
  two halves of the array compute concurrently.
- K bias is dropped (softmax-invariant); V bias is applied on the host as
  bv @ Wo + bo (attention rows sum to 1); V is projected directly in
  [key, head-dim] layout (x^T chunks stationary) so no PE transposes.
- Output partials are fp16 (halves the output DMA).
"""

import numpy as np
import ml_dtypes

import concourse.bacc as bacc
import concourse.mybir as mybir
import concourse.tile as tile
from concourse.bass_utils import run_bass_kernel_spmd

B, S, D, H = 2, 2048, 1024, 16
HD = D // H                 # 64
N_CORES = 8
HPC = H // (N_CORES // B)   # 4 heads per core
HG = HPC * HD               # 256

F32 = mybir.dt.float32
F16 = mybir.dt.float16
BF16 = mybir.dt.bfloat16
AF = mybir.ActivationFunctionType
MUL = mybir.AluOpType.mult
P = 128

NDK = D // P      # 8 contraction tiles for projections
QC = 512          # q-chunk / free-dim grain
NQ = S // QC      # 4
NKT = S // P      # 16 key tiles
KCH = NKT // NQ   # 4 key tiles per 512-key chunk

LN2 = float(np.log(2.0))
A128 = 128.0 / LN2          # folded into Wq host-side
B0 = 127.0 * 128.0
EXP_C0 = float(np.float32(2.0 ** 30 + B0))
EXP_C1 = 0.9870150545644067
EXP_C2 = 0.0026038601994592186
EXP_K = float(np.float32(2.0 ** 23 + B0 - 128.0 + 53.0))

_CACHE = {}


def _register_exp_op():
    """Define + register the custom DVE exp-bits op (idempotent)."""
    import concourse.dve_ops as dops
    from concourse.dve_spec import (
        Spec, Src0, C0, C1, C2, C3, _spill_c3_to_src1, lower,
    )
    from concourse.dve_uop import DveOpSpec

    for op in dops.OPS:
        if op.name == "EXP_BITS_ANT":
            return op

    t1 = Src0 + C0
    d = C0 - t1
    fr = Src0 + d
    p = (fr * C2 + C1) * fr
    u = C3 - d
    body = _spill_c3_to_src1(u + p)

    def _ref(in0, in1, s0, s1, imm2):
        f = np.float32
        w = np.asarray(in0, f)
        t1 = f(w + f(s0))
        dd = f(f(s0) - t1)
        fr = f(w + dd)
        h = f(fr * f(imm2))
        h2 = f(h + f(s1))
        pp = f(h2 * fr)
        uu = f(np.asarray(in1, f) - dd)
        return f(uu + pp)

    spec = Spec(body=body, reference=_ref)
    opcode = dops._CUSTOM_DVE_ROW_BASE + len(dops.OPS)
    sha = {}
    for ver in ("v3",):
        s = DveOpSpec(name="EXP_BITS_ANT", opcode=opcode,
                      uops=lower(spec, ver=ver), rd1_en=True)
        sha[ver] = s.sha(ver)
    op = dops.DveOp("EXP_BITS_ANT", spec, subdim=False, uops_sha=sha)
    dops.OPS.append(op)
    dops._SUB_OPCODE_FOR_NAME[op.name] = opcode
    dops.CUSTOM_DVE_SPECS[op.name] = spec
    return op


def _build():
    nc = bacc.Bacc("TRN2", target_bir_lowering=False, debug=False,
                   num_devices=N_CORES)
    exp_op = _register_exp_op()

    qt_d = nc.dram_tensor("qt", [D, S], BF16, kind="ExternalInput")
    kt_d = nc.dram_tensor("kt", [D, S], BF16, kind="ExternalInput")
    vt_d = nc.dram_tensor("vt", [D, S], BF16, kind="ExternalInput")
    wq_d = nc.dram_tensor("wq", [D, HG], BF16, kind="ExternalInput")
    wk_d = nc.dram_tensor("wk", [D, HG], BF16, kind="ExternalInput")
    wv_d = nc.dram_tensor("wv", [D, HG], BF16, kind="ExternalInput")
    wo_d = nc.dram_tensor("wo", [HG, D], BF16, kind="ExternalInput")
    bq_d = nc.dram_tensor("bq", [HG, 1], F32, kind="ExternalInput")
    out_d = nc.dram_tensor("outT", [D, S], F16, kind="ExternalOutput")

    # static engine-load ledger for exp/evac assignment
    busy = {"s": 0.0, "v": 0.0}

    def pick(cost_s, cost_v):
        if busy["s"] + cost_s <= busy["v"] + cost_v:
            busy["s"] += cost_s
            return "s"
        busy["v"] += cost_v
        return "v"

    def led_copy(dst, src, cost_s=660.0, cost_v=600.0):
        if pick(cost_s, cost_v) == "s":
            nc.scalar.copy(dst, src)
        else:
            nc.vector.tensor_copy(dst, src)

    dmaq = [0]

    def dma(dst, src, engs=None):
        engs = engs or (nc.sync, nc.gpsimd)
        eng = engs[dmaq[0] % len(engs)]
        dmaq[0] += 1
        eng.dma_start(dst, src)

    with tile.TileContext(nc) as tc:
        with (
            tc.tile_pool(name="persist", bufs=1) as pp,
            tc.tile_pool(name="ps_pool", bufs=2, space="PSUM") as psp,
            tc.tile_pool(name="po_pool", bufs=4, space="PSUM") as pop,
            tc.tile_pool(name="ptb_pool", bufs=34) as ptb,
            tc.tile_pool(name="ptf_pool", bufs=11) as ptf,
            tc.tile_pool(name="iox", bufs=3) as iop,
            tc.tile_pool(name="ot_pool", bufs=4) as otp,
            tc.tile_pool(name="rc_pool", bufs=2) as rcp,
            tc.tile_pool(name="at_pool", bufs=4) as atp,
        ):
            qt_sb = [pp.tile([P, S], BF16, name=f"qt_sb{m}") for m in range(2)]
            kt_sb = [pp.tile([P, S], BF16, name=f"kt_sb{m}") for m in range(2)]
            vb = pp.tile([P, NKT, HPC, 2 * HD], BF16)
            wq_sb = pp.tile([P, NDK, HG], BF16)
            wk_sb = pp.tile([P, NDK, HG], BF16)
            wv_sb = pp.tile([P, NDK, HG], BF16)
            wo_sb = pp.tile([P, 2, D], BF16)
            bq_sb = pp.tile([P, 2], F32)
            kp_sb = pp.tile([P, 1], F32)
            bexp_sb = pp.tile([P, 1], F32)

            # weights in dk-pairs spread across the 3 DMA-capable engines;
            # emitted just-in-time per phase (consumers gate on cumulative
            # DMA semaphore counts, so late weights would stall early MMs)
            E4 = (nc.sync, nc.gpsimd, nc.scalar)

            def dma_w(w_sb, w_d):
                for h in range(NDK // 2):
                    dma(w_sb[:, 2 * h:2 * h + 2, :],
                        w_d[2 * h * P:(2 * h + 2) * P, :]
                        .rearrange("(a p) n -> p a n", p=P), engs=E4)

            nc.vector.memset(kp_sb[:], EXP_K)
            nc.vector.memset(bexp_sb[:], -LN2 / 2.0)
            nc.gpsimd.memset(vb[:, :, :, HD:2 * HD], 1.0)
            busy["v"] += 1000.0

            def pe_warm(n):
                # tiny no-dep matmuls absorb PE starvation windows so the
                # HAM clock gate stays at 8/8 through the DMA fill
                wps = pop.tile([1, 1], F32, tag="po", name="warm_ps",
                               padded_shape=[P, QC])
                for _ in range(n):
                    nc.tensor.matmul(wps[:], bexp_sb[:, 0:1],
                                     kp_sb[:, 0:1], start=True, stop=True)

            def stage_x(dram, qc, engs=None):
                xs = []
                for dk in range(NDK):
                    xt = iop.tile([P, QC], BF16, tag=f"x{dk}", name=f"x{dk}")
                    dma(xt[:], dram[dk * P:(dk + 1) * P,
                                    qc * QC:(qc + 1) * QC], engs=engs)
                    xs.append(xt)
                return lambda dk: xs[dk][:]

            # ---------- exp + score helpers ----------
            pts = {}   # slot t -> list of (ptile, engine) per kt

            def emit_score_kt(t, kt):
                qj, mh = divmod(t, 2)
                qs = slice(qj * QC, (qj + 1) * QC)
                kc = slice(kt * P, (kt + 1) * P)
                ps = psp.tile([P, 2 * QC], F32, tag="ps", name="s_ps")
                nc.tensor.matmul(ps[:, 0:QC], kt_sb[mh][0:HD, kc],
                                 qt_sb[mh][0:HD, qs], start=True, stop=True)
                nc.tensor.matmul(ps[:, QC:2 * QC], kt_sb[mh][HD:P, kc],
                                 qt_sb[mh][HD:P, qs], start=True, stop=True)
                if pick(1085.0, 1140.0) == "s":
                    pt = ptb.tile([P, 2 * QC], BF16, tag="ptb", name="ptb")
                    nc.scalar.activation(pt[:], ps[:], AF.Exp,
                                         bias=bexp_sb[:, 0:1], scale=LN2 / 128.0)
                    eng = "s"
                else:
                    pt = ptf.tile([P, 2 * QC], F32, tag="ptf", name="ptf")
                    nc.vector._custom_dve(exp_op, out=pt[:], in0=ps[:],
                                          in1=kp_sb[:, 0:1], s0=EXP_C0,
                                          s1=EXP_C1, imm2=EXP_C2)
                    eng = "v"
                pts.setdefault(t, []).append((pt, eng))

            def emit_S(t):
                for kt in range(NKT):
                    emit_score_kt(t, kt)

            def pt_rhs(pt, eng, h2):
                if eng == "s":
                    return pt[:, h2 * QC:(h2 + 1) * QC]
                v = pt[:].bitcast(BF16)  # [P, 4*QC], even u16 lanes hold bf16
                sl = v[:, 2 * h2 * QC:2 * (h2 + 1) * QC]
                return sl.rearrange("p (n t) -> p n t", t=2)[:, :, 0:1]

            at_tiles = {}

            def emit_PV_fin(t):
                qj, mh = divmod(t, 2)
                po_e = pop.tile([P, QC], F32, tag="po", name="po_e")
                po_o = pop.tile([P, QC], F32, tag="po", name="po_o")
                for kt in range(NKT):
                    pt, eng = pts[t][kt]
                    st = (kt == 0)
                    sp = (kt == NKT - 1)
                    nc.tensor.matmul(po_e[:], vb[:, kt, 2 * mh + 0, :],
                                     pt_rhs(pt, eng, 0), start=st, stop=sp)
                    nc.tensor.matmul(po_o[:], vb[:, kt, 2 * mh + 1, :],
                                     pt_rhs(pt, eng, 1), start=st, stop=sp)
                del pts[t]
                att = atp.tile([P, QC], BF16, tag="at", name="att")
                for h2, po in ((0, po_e), (1, po_o)):
                    rs = rcp.tile([HD, QC], F32, tag="rs", name="rs")
                    nc.scalar.copy(rs[:], po[HD:P, :])
                    rc = rcp.tile([HD, QC], F32, tag="rc", name="rc")
                    nc.vector.reciprocal_approx_fast(rc[:], rs[:])
                    nc.vector.tensor_tensor(att[h2 * HD:(h2 + 1) * HD, :],
                                            po[0:HD, :], rc[:], MUL)
                at_tiles[(qj, mh)] = att

            def emit_OP(qj):
                qs = slice(qj * QC, (qj + 1) * QC)
                for dg in range(8):
                    cols = slice(dg * P, (dg + 1) * P)
                    pso = pop.tile([P, QC], F32, tag="po", name="op_ps")
                    nc.tensor.matmul(pso[:], wo_sb[:, 0, cols],
                                     at_tiles[(qj, 0)][:], start=True, stop=False)
                    nc.tensor.matmul(pso[:], wo_sb[:, 1, cols],
                                     at_tiles[(qj, 1)][:], start=False, stop=True)
                    ot = otp.tile([P, QC], F16, tag="ot", name="ot")
                    led_copy(ot[:], pso[:])
                    dma(out_d[cols, qs], ot[:])

            # ---------- Phase A/B merged: per 512-chunk, project Q (chunk
            # qc), K, V, then run qj=0 scores + exp for this chunk's key
            # tiles (they only need Q chunk 0, done first) ----------
            for qc in range(NQ):
                qcs = slice(qc * QC, (qc + 1) * QC)
                if qc == 0:
                    dma_w(wq_sb, wq_d)
                xq = stage_x(qt_d, qc, engs=E4 if qc == 0 else None)
                if qc == 0:
                    dma(bq_sb[:], bq_d[:].rearrange("(a p) o -> p (a o)", p=P),
                        engs=E4)
                    pe_warm(72)
                for m in range(2):
                    ps = pop.tile([P, QC], F32, tag="po", name="qproj_ps")
                    cols = slice(m * P, (m + 1) * P)
                    for dk in range(NDK):
                        nc.tensor.matmul(ps[:], wq_sb[:, dk, cols], xq(dk),
                                         start=(dk == 0), stop=(dk == NDK - 1))
                    nc.vector.tensor_scalar_add(qt_sb[m][:, qcs], ps[:],
                                                bq_sb[:, m:m + 1])
                    busy["v"] += 600.0
                if qc == 0:
                    dma_w(wk_sb, wk_d)
                xk = stage_x(kt_d, qc, engs=E4 if qc == 0 else None)
                for m in range(2):
                    ps = pop.tile([P, QC], F32, tag="po", name="kproj_ps")
                    cols = slice(m * P, (m + 1) * P)
                    for dk in range(NDK):
                        nc.tensor.matmul(ps[:], wk_sb[:, dk, cols], xk(dk),
                                         start=(dk == 0), stop=(dk == NDK - 1))
                    led_copy(kt_sb[m][:, qcs], ps[:])

                def vproj(vc):
                    xv = stage_x(vt_d, vc)
                    for st in range(KCH):
                        kt = vc * KCH + st
                        ps2 = pop.tile([P, HG], F32, tag="po", name="vproj_ps",
                                       padded_shape=[P, QC])
                        scol = slice(st * P, (st + 1) * P)
                        for dk in range(NDK):
                            nc.tensor.matmul(ps2[:], xv(dk)[:, scol],
                                             wv_sb[:, dk, :],
                                             start=(dk == 0), stop=(dk == NDK - 1))
                        nc.vector.tensor_copy(
                            vb[:, kt, :, 0:HD],
                            ps2[:].rearrange("p (h d) -> p h d", h=HPC))
                        busy["v"] += 330.0

                # V-proj deferred one chunk: keeps its input DMA out of the
                # startup fill window (it isn't consumed until phase C)
                if qc == 1:
                    dma_w(wv_sb, wv_d)
                    dma(wo_sb[:, 0, :], wo_d[0:P, :])
                    dma(wo_sb[:, 1, :], wo_d[P:2 * P, :])
                if qc > 0:
                    vproj(qc - 1)
                # qj=0 scores for this chunk's key tiles
                for mh in range(2):
                    for st in range(KCH):
                        emit_score_kt(mh, qc * KCH + st)
                if qc == NQ - 1:
                    vproj(qc)

            for t in (0, 1):
                assert len(pts[t]) == NKT

            # ---------- Phase C: attention slots ----------
            for t in range(2 * NQ):
                # pre-charge this window's fixed fin/OP engine costs so the
                # S(t+2) exp assignment accounts for them (they execute in
                # the same time window but are emitted later)
                busy["s"] += 1040.0
                busy["v"] += 2400.0
                if t + 2 <= 2 * NQ - 1:
                    emit_S(t + 2)
                emit_PV_fin(t)
                if t % 2 == 1:
                    emit_OP(t // 2)

    nc.compile()
    return nc


def kernel(query, key, value, Wq, bq, Wk, bk, Wv, bv, Wo, bo):
    if "nc" not in _CACHE:
        _CACHE["nc"] = _build()
    nc = _CACHE["nc"]

    bf = ml_dtypes.bfloat16
    scale = np.float32(A128 / np.sqrt(HD))
    xt = {}
    for b in range(B):
        xt[("q", b)] = np.ascontiguousarray(query[b].T).astype(bf)
        xt[("k", b)] = np.ascontiguousarray(key[b].T).astype(bf)
        xt[("v", b)] = np.ascontiguousarray(value[b].T).astype(bf)

    in_maps = []
    for c in range(N_CORES):
        b, g = c // (N_CORES // B), c % (N_CORES // B)
        cols = slice(g * HG, (g + 1) * HG)
        in_maps.append({
            "qt": xt[("q", b)],
            "kt": xt[("k", b)],
            "vt": xt[("v", b)],
            "wq": (np.ascontiguousarray(Wq[:, cols]) * scale).astype(bf),
            "wk": np.ascontiguousarray(Wk[:, cols]).astype(bf),
            "wv": np.ascontiguousarray(Wv[:, cols]).astype(bf),
            "wo": np.ascontiguousarray(Wo[cols, :]).astype(bf),
            "bq": (bq[cols] * scale).reshape(HG, 1).astype(np.float32),
        })

    global _last_in_maps
    _last_in_maps = in_maps
    res = run_bass_kernel_spmd(nc, in_maps, list(range(N_CORES)))

    out = np.zeros((B, S, D), dtype=np.float32)
    for c in range(N_CORES):
        b = c // (N_CORES // B)
        out[b] += res.results[c]["outT"].astype(np.float32).T
    out += (bv.astype(np.float32) @ Wo.astype(np.float32) + bo).astype(np.float32)
    return out


# revision 35
# speedup vs baseline: 1.1534x; 1.1534x over previous
"""Multi-head attention (B=2, S=2048, D=1024, H=16) on 8 TRN2 NeuronCores.

Sharding: core c handles batch b = c // 4 and heads 4*(c%4) .. 4*(c%4)+3.
Each core computes its 4 heads' Q/K/V projections, head-local attention,
and a partial output projection (row slice of Wo). Host sums the partials.

Key points vs the fp16 baseline:
- Everything bf16. Scores are computed pre-scaled by 128/ln2 (folded into
  Wq host-side) so both exp engines work in "bf16 index space".
- Softmax exp runs on TWO engines: ScalarE does true exp (free affine
  bias/scale) while the DVE runs a custom 8-slice op that emits the
  bf16 BIT PATTERN of exp(x)*2^-1/2 in the low half of an fp32 write
  (floor-split at the 128-index grain + quadratic mantissa poly). The
  global 2^-1/2 factor is softmax-invariant. The PV matmul reads DVE
  tiles through a stride-2 bf16 view.
- Score matmuls (contraction=64) are issued as back-to-back even/odd-head
  pairs on PE row tiles (0,0)/(64,0) writing different PSUM banks, so the
  two halves of the array compute concurrently.
- K bias is dropped (softmax-invariant); V bias is applied on the host as
  bv @ Wo + bo (attention rows sum to 1); V is projected directly in
  [key, head-dim] layout (x^T chunks stationary) so no PE transposes.
- Output partials are fp16 (halves the output DMA).
"""

import numpy as np
import ml_dtypes

import concourse.bacc as bacc
import concourse.mybir as mybir
import concourse.tile as tile
from concourse.bass_utils import run_bass_kernel_spmd

B, S, D, H = 2, 2048, 1024, 16
HD = D // H                 # 64
N_CORES = 8
HPC = H // (N_CORES // B)   # 4 heads per core
HG = HPC * HD               # 256

F32 = mybir.dt.float32
F16 = mybir.dt.float16
BF16 = mybir.dt.bfloat16
AF = mybir.ActivationFunctionType
MUL = mybir.AluOpType.mult
P = 128

NDK = D // P      # 8 contraction tiles for projections
QC = 512          # q-chunk / free-dim grain
NQ = S // QC      # 4
NKT = S // P      # 16 key tiles
KCH = NKT // NQ   # 4 key tiles per 512-key chunk

LN2 = float(np.log(2.0))
A128 = 128.0 / LN2          # folded into Wq host-side
B0 = 127.0 * 128.0
EXP_C0 = float(np.float32(2.0 ** 30 + B0))
EXP_C1 = 0.9870150545644067
EXP_C2 = 0.0026038601994592186
EXP_K = float(np.float32(2.0 ** 23 + B0 - 128.0 + 53.0))

_CACHE = {}


def _register_exp_op():
    """Define + register the custom DVE exp-bits op (idempotent)."""
    import concourse.dve_ops as dops
    from concourse.dve_spec import (
        Spec, Src0, C0, C1, C2, C3, _spill_c3_to_src1, lower,
    )
    from concourse.dve_uop import DveOpSpec

    for op in dops.OPS:
        if op.name == "EXP_BITS_ANT":
            return op

    t1 = Src0 + C0
    d = C0 - t1
    fr = Src0 + d
    p = (fr * C2 + C1) * fr
    u = C3 - d
    body = _spill_c3_to_src1(u + p)

    def _ref(in0, in1, s0, s1, imm2):
        f = np.float32
        w = np.asarray(in0, f)
        t1 = f(w + f(s0))
        dd = f(f(s0) - t1)
        fr = f(w + dd)
        h = f(fr * f(imm2))
        h2 = f(h + f(s1))
        pp = f(h2 * fr)
        uu = f(np.asarray(in1, f) - dd)
        return f(uu + pp)

    spec = Spec(body=body, reference=_ref)
    opcode = dops._CUSTOM_DVE_ROW_BASE + len(dops.OPS)
    sha = {}
    for ver in ("v3",):
        s = DveOpSpec(name="EXP_BITS_ANT", opcode=opcode,
                      uops=lower(spec, ver=ver), rd1_en=True)
        sha[ver] = s.sha(ver)
    op = dops.DveOp("EXP_BITS_ANT", spec, subdim=False, uops_sha=sha)
    dops.OPS.append(op)
    dops._SUB_OPCODE_FOR_NAME[op.name] = opcode
    dops.CUSTOM_DVE_SPECS[op.name] = spec
    return op


def _build():
    nc = bacc.Bacc("TRN2", target_bir_lowering=False, debug=False,
                   num_devices=N_CORES)
    exp_op = _register_exp_op()

    qt_d = nc.dram_tensor("qt", [D, S], BF16, kind="ExternalInput")
    kt_d = nc.dram_tensor("kt", [D, S], BF16, kind="ExternalInput")
    vt_d = nc.dram_tensor("vt", [D, S], BF16, kind="ExternalInput")
    wq_d = nc.dram_tensor("wq", [D, HG], BF16, kind="ExternalInput")
    wk_d = nc.dram_tensor("wk", [D, HG], BF16, kind="ExternalInput")
    wv_d = nc.dram_tensor("wv", [D, HG], BF16, kind="ExternalInput")
    wo_d = nc.dram_tensor("wo", [HG, D], BF16, kind="ExternalInput")
    bq_d = nc.dram_tensor("bq", [HG, 1], F32, kind="ExternalInput")
    out_d = nc.dram_tensor("outT", [D, S], F16, kind="ExternalOutput")

    # static engine-load ledger for exp/evac assignment
    busy = {"s": 0.0, "v": 0.0}

    def pick(cost_s, cost_v):
        if busy["s"] + cost_s <= busy["v"] + cost_v:
            busy["s"] += cost_s
            return "s"
        busy["v"] += cost_v
        return "v"

    def led_copy(dst, src, cost_s=660.0, cost_v=600.0):
        if pick(cost_s, cost_v) == "s":
            nc.scalar.copy(dst, src)
        else:
            nc.vector.tensor_copy(dst, src)

    dmaq = [0]

    def dma(dst, src, engs=None):
        engs = engs or (nc.sync, nc.gpsimd)
        eng = engs[dmaq[0] % len(engs)]
        dmaq[0] += 1
        eng.dma_start(dst, src)

    with tile.TileContext(nc) as tc:
        with (
            tc.tile_pool(name="persist", bufs=1) as pp,
            tc.tile_pool(name="ps_pool", bufs=2, space="PSUM") as psp,
            tc.tile_pool(name="po_pool", bufs=4, space="PSUM") as pop,
            tc.tile_pool(name="ptb_pool", bufs=34) as ptb,
            tc.tile_pool(name="ptf_pool", bufs=11) as ptf,
            tc.tile_pool(name="iox", bufs=3) as iop,
            tc.tile_pool(name="ot_pool", bufs=4) as otp,
            tc.tile_pool(name="rc_pool", bufs=2) as rcp,
            tc.tile_pool(name="at_pool", bufs=4) as atp,
        ):
            qt_sb = [pp.tile([P, S], BF16, name=f"qt_sb{m}") for m in range(2)]
            kt_sb = [pp.tile([P, S], BF16, name=f"kt_sb{m}") for m in range(2)]
            vb = pp.tile([P, NKT, HPC, 2 * HD], BF16)
            wq_sb = pp.tile([P, NDK, HG], BF16)
            wk_sb = pp.tile([P, NDK, HG], BF16)
            wv_sb = pp.tile([P, NDK, HG], BF16)
            wo_sb = pp.tile([P, 2, D], BF16)
            bq_sb = pp.tile([P, 2], F32)
            kp_sb = pp.tile([P, 1], F32)
            bexp_sb = pp.tile([P, 1], F32)

            # weights in dk-pairs spread across the 3 DMA-capable engines;
            # emitted just-in-time per phase (consumers gate on cumulative
            # DMA semaphore counts, so late weights would stall early MMs)
            E4 = (nc.sync, nc.gpsimd, nc.scalar)

            def dma_w(w_sb, w_d):
                for h in range(NDK // 2):
                    dma(w_sb[:, 2 * h:2 * h + 2, :],
                        w_d[2 * h * P:(2 * h + 2) * P, :]
                        .rearrange("(a p) n -> p a n", p=P), engs=E4)

            nc.vector.memset(kp_sb[:], EXP_K)
            nc.vector.memset(bexp_sb[:], -LN2 / 2.0)
            nc.gpsimd.memset(vb[:, :, :, HD:2 * HD], 1.0)
            busy["v"] += 1000.0

            def pe_warm(n):
                # tiny no-dep matmuls absorb PE starvation windows so the
                # HAM clock gate stays at 8/8 through the DMA fill
                wps = pop.tile([1, 1], F32, tag="po", name="warm_ps",
                               padded_shape=[P, QC])
                for _ in range(n):
                    nc.tensor.matmul(wps[:], bexp_sb[:, 0:1],
                                     kp_sb[:, 0:1], start=True, stop=True)

            def stage_x(dram, qc, engs=None):
                xs = []
                for dk in range(NDK):
                    xt = iop.tile([P, QC], BF16, tag=f"x{dk}", name=f"x{dk}")
                    dma(xt[:], dram[dk * P:(dk + 1) * P,
                                    qc * QC:(qc + 1) * QC], engs=engs)
                    xs.append(xt)
                return lambda dk: xs[dk][:]

            # ---------- exp + score helpers ----------
            pts = {}   # slot t -> list of (ptile, engine) per kt

            def emit_score_kt(t, kt):
                qj, mh = divmod(t, 2)
                qs = slice(qj * QC, (qj + 1) * QC)
                kc = slice(kt * P, (kt + 1) * P)
                ps = psp.tile([P, 2 * QC], F32, tag="ps", name="s_ps")
                nc.tensor.matmul(ps[:, 0:QC], kt_sb[mh][0:HD, kc],
                                 qt_sb[mh][0:HD, qs], start=True, stop=True)
                nc.tensor.matmul(ps[:, QC:2 * QC], kt_sb[mh][HD:P, kc],
                                 qt_sb[mh][HD:P, qs], start=True, stop=True)
                if pick(1085.0, 1140.0) == "s":
                    pt = ptb.tile([P, 2 * QC], BF16, tag="ptb", name="ptb")
                    nc.scalar.activation(pt[:], ps[:], AF.Exp,
                                         bias=bexp_sb[:, 0:1], scale=LN2 / 128.0)
                    eng = "s"
                else:
                    pt = ptf.tile([P, 2 * QC], F32, tag="ptf", name="ptf")
                    nc.vector._custom_dve(exp_op, out=pt[:], in0=ps[:],
                                          in1=kp_sb[:, 0:1], s0=EXP_C0,
                                          s1=EXP_C1, imm2=EXP_C2)
                    eng = "v"
                pts.setdefault(t, []).append((pt, eng))

            def emit_S(t):
                for kt in range(NKT):
                    emit_score_kt(t, kt)

            def pt_rhs(pt, eng, h2):
                if eng == "s":
                    return pt[:, h2 * QC:(h2 + 1) * QC]
                v = pt[:].bitcast(BF16)  # [P, 4*QC], even u16 lanes hold bf16
                sl = v[:, 2 * h2 * QC:2 * (h2 + 1) * QC]
                return sl.rearrange("p (n t) -> p n t", t=2)[:, :, 0:1]

            at_tiles = {}

            def emit_PV_fin(t):
                qj, mh = divmod(t, 2)
                po_e = pop.tile([P, QC], F32, tag="po", name="po_e")
                po_o = pop.tile([P, QC], F32, tag="po", name="po_o")
                for kt in range(NKT):
                    pt, eng = pts[t][kt]
                    st = (kt == 0)
                    sp = (kt == NKT - 1)
                    nc.tensor.matmul(po_e[:], vb[:, kt, 2 * mh + 0, :],
                                     pt_rhs(pt, eng, 0), start=st, stop=sp)
                    nc.tensor.matmul(po_o[:], vb[:, kt, 2 * mh + 1, :],
                                     pt_rhs(pt, eng, 1), start=st, stop=sp)
                del pts[t]
                att = atp.tile([P, QC], BF16, tag="at", name="att")
                for h2, po in ((0, po_e), (1, po_o)):
                    rs = rcp.tile([HD, QC], F32, tag="rs", name="rs")
                    nc.scalar.copy(rs[:], po[HD:P, :])
                    rc = rcp.tile([HD, QC], F32, tag="rc", name="rc")
                    nc.vector.reciprocal_approx_fast(rc[:], rs[:])
                    nc.vector.tensor_tensor(att[h2 * HD:(h2 + 1) * HD, :],
                                            po[0:HD, :], rc[:], MUL)
                at_tiles[(qj, mh)] = att

            def emit_OP(qj):
                qs = slice(qj * QC, (qj + 1) * QC)
                for dg in range(8):
                    cols = slice(dg * P, (dg + 1) * P)
                    pso = pop.tile([P, QC], F32, tag="po", name="op_ps")
                    nc.tensor.matmul(pso[:], wo_sb[:, 0, cols],
                                     at_tiles[(qj, 0)][:], start=True, stop=False)
                    nc.tensor.matmul(pso[:], wo_sb[:, 1, cols],
                                     at_tiles[(qj, 1)][:], start=False, stop=True)
                    ot = otp.tile([P, QC], F16, tag="ot", name="ot")
                    led_copy(ot[:], pso[:])
                    dma(out_d[cols, qs], ot[:])

            # ---------- Phase A/B merged: per 512-chunk, project Q (chunk
            # qc), K, V, then run qj=0 scores + exp for this chunk's key
            # tiles (they only need Q chunk 0, done first) ----------
            for qc in range(NQ):
                qcs = slice(qc * QC, (qc + 1) * QC)
                if qc == 0:
                    dma_w(wq_sb, wq_d)
                xq = stage_x(qt_d, qc, engs=E4 if qc == 0 else None)
                if qc == 0:
                    dma(bq_sb[:], bq_d[:].rearrange("(a p) o -> p (a o)", p=P),
                        engs=E4)
                    pe_warm(72)
                for m in range(2):
                    ps = pop.tile([P, QC], F32, tag="po", name="qproj_ps")
                    cols = slice(m * P, (m + 1) * P)
                    for dk in range(NDK):
                        nc.tensor.matmul(ps[:], wq_sb[:, dk, cols], xq(dk),
                                         start=(dk == 0), stop=(dk == NDK - 1))
                    nc.vector.tensor_scalar_add(qt_sb[m][:, qcs], ps[:],
                                                bq_sb[:, m:m + 1])
                    busy["v"] += 600.0
                if qc == 0:
                    dma_w(wk_sb, wk_d)
                xk = stage_x(kt_d, qc, engs=E4 if qc == 0 else None)
                for m in range(2):
                    ps = pop.tile([P, QC], F32, tag="po", name="kproj_ps")
                    cols = slice(m * P, (m + 1) * P)
                    for dk in range(NDK):
                        nc.tensor.matmul(ps[:], wk_sb[:, dk, cols], xk(dk),
                                         start=(dk == 0), stop=(dk == NDK - 1))
                    led_copy(kt_sb[m][:, qcs], ps[:])
                if qc == 0:
                    dma_w(wv_sb, wv_d)
                xv = stage_x(vt_d, qc, engs=E4 if qc == 0 else None)
                for st in range(KCH):
                    kt = qc * KCH + st
                    ps2 = pop.tile([P, HG], F32, tag="po", name="vproj_ps",
                                   padded_shape=[P, QC])
                    scol = slice(st * P, (st + 1) * P)
                    for dk in range(NDK):
                        nc.tensor.matmul(ps2[:], xv(dk)[:, scol], wv_sb[:, dk, :],
                                         start=(dk == 0), stop=(dk == NDK - 1))
                    nc.vector.tensor_copy(
                        vb[:, kt, :, 0:HD],
                        ps2[:].rearrange("p (h d) -> p h d", h=HPC))
                    busy["v"] += 330.0
                if qc == 0:
                    dma(wo_sb[:, 0, :], wo_d[0:P, :])
                    dma(wo_sb[:, 1, :], wo_d[P:2 * P, :])
                # qj=0 scores for this chunk's key tiles
                for mh in range(2):
                    for st in range(KCH):
                        emit_score_kt(mh, qc * KCH + st)

            for t in (0, 1):
                assert len(pts[t]) == NKT

            # ---------- Phase C: attention slots ----------
            for t in range(2 * NQ):
                # pre-charge this window's fixed fin/OP engine costs so the
                # S(t+2) exp assignment accounts for them (they execute in
                # the same time window but are emitted later)
                busy["s"] += 1040.0
                busy["v"] += 2400.0
                if t + 2 <= 2 * NQ - 1:
                    emit_S(t + 2)
                emit_PV_fin(t)
                if t % 2 == 1:
                    emit_OP(t // 2)

    nc.compile()
    return nc


def kernel(query, key, value, Wq, bq, Wk, bk, Wv, bv, Wo, bo):
    if "nc" not in _CACHE:
        _CACHE["nc"] = _build()
    nc = _CACHE["nc"]

    bf = ml_dtypes.bfloat16
    scale = np.float32(A128 / np.sqrt(HD))
    xt = {}
    for b in range(B):
        xt[("q", b)] = np.ascontiguousarray(query[b].T).astype(bf)
        xt[("k", b)] = np.ascontiguousarray(key[b].T).astype(bf)
        xt[("v", b)] = np.ascontiguousarray(value[b].T).astype(bf)

    in_maps = []
    for c in range(N_CORES):
        b, g = c // (N_CORES // B), c % (N_CORES // B)
        cols = slice(g * HG, (g + 1) * HG)
        in_maps.append({
            "qt": xt[("q", b)],
            "kt": xt[("k", b)],
            "vt": xt[("v", b)],
            "wq": (np.ascontiguousarray(Wq[:, cols]) * scale).astype(bf),
            "wk": np.ascontiguousarray(Wk[:, cols]).astype(bf),
            "wv": np.ascontiguousarray(Wv[:, cols]).astype(bf),
            "wo": np.ascontiguousarray(Wo[cols, :]).astype(bf),
            "bq": (bq[cols] * scale).reshape(HG, 1).astype(np.float32),
        })

    global _last_in_maps
    _last_in_maps = in_maps
    res = run_bass_kernel_spmd(nc, in_maps, list(range(N_CORES)))

    out = np.zeros((B, S, D), dtype=np.float32)
    for c in range(N_CORES):
        b = c // (N_CORES // B)
        out[b] += res.results[c]["outT"].astype(np.float32).T
    out += (bv.astype(np.float32) @ Wo.astype(np.float32) + bo).astype(np.float32)
    return out
